# revision 14
# baseline (speedup 1.0000x reference)
"""Trainium2 Bass kernel for the CARP3D attention-MIL pooling model.

Model (per bag b of B=16, N=8192 instances, F=512 features):
    h1 = relu(h @ W1 + b1)            [B,N,H]   H=512
    a  = tanh(h1 @ Wa + ba)           [B,N,D]   D=256
    g  = sigmoid(h1 @ Wb + bb)        [B,N,D]
    A  = (a*g) @ Wc + bc              [B,N,1]
    A_sm = softmax(A over N)
    M  = A_sm @ h1                    [B,1,H]
    context = mean_b M; logits = context @ Wcls + bcls

Distribution: data-parallel over bags, 2 bags per core on 8 cores.

Layout strategy: the host pre-transposes h to [bag, F, N] bf16 so the
contraction dim (features) is on SBUF partitions for every matmul and no
on-device transposes are needed.  All compute stays in "feature-major"
space:
    h1T[H,n] = relu(W1.T @ hT)       (lhsT = W1 blocks, rhs = hT)
    aT[D,n]  = tanh(Wa.T @ h1T)
    gT       = tanh(0.5*Wb.T @ h1T)  (sigmoid(x) = 0.5*(1+tanh(x/2)))
    s'       = aT + aT*gT            (= 2*a*g)
    A_rep    = (0.5*Wc replicated 128x).T @ s'   -> [128, n] with every
               partition holding the same score row (free broadcast)
    w        = exp(A_rep)            (no max-subtraction: |A| << 1 here)
    S[H]    += sum_n h1T[H,n]*w[n]   (fused DVE tensor_tensor_reduce)
Outputs per core: raw score rows A (f32) and unnormalized pooled S (f32).
The softmax normalization (Z), bc shift (softmax-invariant), mean over
bags and the [1,512]@[512,2] classifier run on host - microseconds of
work vs the 256MB streaming on device.
"""

import os
import sys

import numpy as np
import ml_dtypes

for _p in ("/opt/trn_rl_repo",):
    if _p not in sys.path:
        sys.path.insert(0, _p)

import concourse.bass as bass
import concourse.mybir as mybir
import concourse.tile as tile
from concourse import bacc
from concourse.bass import ts
from concourse.bass_utils import run_bass_kernel_spmd

BF16 = mybir.dt.bfloat16
F32 = mybir.dt.float32

B, N, F, H, D = 16, 8192, 512, 512, 256
NCORES = 8
BAGS_PER_CORE = B // NCORES          # 2
T = 512                              # instance tile size
NT = N // T                          # 16 tiles per bag
FC = F // 128                        # 4 F-chunks
HC = H // 128                        # 4 H-chunks
DC = D // 128                        # 2 D-chunks

# set by test.py to collect a hardware profile
PROFILE = False
LAST_EXEC_NS = None
_CACHE = {}


def _build(use_bias: bool):
    nc = bacc.Bacc("TRN2", target_bir_lowering=False)

    # h layout: [bag, n_tile, partition, f_chunk, n_in_tile] so each
    # partition's per-tile data is one contiguous 4KB run (fast descriptors)
    h_d = nc.dram_tensor("h", [BAGS_PER_CORE, NT, 128, FC, T], BF16, kind="ExternalInput")
    w1_d = nc.dram_tensor("w1", [F, H], BF16, kind="ExternalInput")
    wa_d = nc.dram_tensor("wa", [H, D], BF16, kind="ExternalInput")
    wb2_d = nc.dram_tensor("wb2", [H, D], BF16, kind="ExternalInput")
    wc2r_d = nc.dram_tensor("wc2r", [D, 128], BF16, kind="ExternalInput")
    if use_bias:
        b1_d = nc.dram_tensor("b1r", [1, H], BF16, kind="ExternalInput")
        ba_d = nc.dram_tensor("bar", [1, D], BF16, kind="ExternalInput")
        bb2_d = nc.dram_tensor("bb2r", [1, D], BF16, kind="ExternalInput")
    a_out = nc.dram_tensor("a_out", [BAGS_PER_CORE, N], F32, kind="ExternalOutput")
    # partition-major so the DMA writes 16B runs, not 4B scatter
    s_out = nc.dram_tensor("s_out", [BAGS_PER_CORE, 128, HC], F32, kind="ExternalOutput")

    with tile.TileContext(nc) as tc:
        with (
            tc.tile_pool(name="const", bufs=1) as const,
            tc.tile_pool(name="hin", bufs=4) as hin,
            tc.tile_pool(name="h1pool", bufs=2) as h1pool,
            tc.tile_pool(name="acts", bufs=2) as actp,
            tc.tile_pool(name="wexp", bufs=2) as wexp,
            tc.tile_pool(name="scratch", bufs=1) as scratch,
            tc.tile_pool(name="spart", bufs=2) as spartp,
            tc.tile_pool(name="sacc", bufs=2) as saccp,
            tc.tile_pool(name="arow", bufs=2) as arowp,
            tc.tile_pool(name="h1psum", bufs=1, space="PSUM") as h1psum,
            tc.tile_pool(name="agpsum", bufs=1, space="PSUM") as agpsum,
        ):
            # --- weights, loaded once ---
            w1_sb = const.tile([128, FC, H], BF16)
            nc.sync.dma_start(out=w1_sb, in_=w1_d.rearrange("(kc p) h -> p kc h", p=128))
            wa_sb = const.tile([128, HC, D], BF16)
            nc.sync.dma_start(out=wa_sb, in_=wa_d.rearrange("(kc p) d -> p kc d", p=128))
            wb2_sb = const.tile([128, HC, D], BF16)
            nc.sync.dma_start(out=wb2_sb, in_=wb2_d.rearrange("(kc p) d -> p kc d", p=128))
            wc2r_sb = const.tile([128, DC, 128], BF16)
            nc.sync.dma_start(out=wc2r_sb, in_=wc2r_d.rearrange("(kc p) m -> p kc m", p=128))
            if use_bias:
                b1_sb = const.tile([1, H], BF16)
                nc.sync.dma_start(out=b1_sb, in_=b1_d)
                ba_sb = const.tile([1, D], BF16)
                nc.sync.dma_start(out=ba_sb, in_=ba_d)
                bb2_sb = const.tile([1, D], BF16)
                nc.sync.dma_start(out=bb2_sb, in_=bb2_d)
                ones_sb = const.tile([1, T], BF16)
                nc.vector.memset(ones_sb, 1.0)

            for bag in range(BAGS_PER_CORE):
                s_acc = saccp.tile([128, HC], F32)
                a_row = arowp.tile([1, N], F32)

                # score path of tile t-1, emitted during tile t so exp /
                # a_row / pooling hide behind tile t's h1 matmuls and the
                # ag psum slot is free again before tile t's ag matmuls
                def score_path(prev, s_acc=s_acc, a_row=a_row):
                    pt, p_ag, p_s, p_h1 = prev
                    for kc in range(DC):
                        nc.tensor.matmul(
                            p_ag[:, 0, :],
                            lhsT=wc2r_sb[:, kc, :],
                            rhs=p_s[:, kc, :],
                            start=(kc == 0),
                            stop=(kc == DC - 1),
                        )
                    w_sb = wexp.tile([128, T], BF16)
                    nc.scalar.activation(w_sb, p_ag[:, 0, :], mybir.ActivationFunctionType.Exp)
                    nc.vector.tensor_copy(out=a_row[0:1, ts(pt, T)], in_=p_ag[0:1, 0, :])
                    wh = scratch.tile([128, HC, T], BF16)
                    w_bc2 = bass.AP(
                        tensor=w_sb.tensor,
                        offset=w_sb.offset,
                        ap=[w_sb.ap[0], [0, 2], w_sb.ap[1]],
                    )
                    nc.vector.tensor_mul(wh[:, 0:2, :], p_h1[0], w_bc2)
                    nc.vector.tensor_mul(wh[:, 2:4, :], p_h1[1], w_bc2)
                    s_part = spartp.tile([128, HC], F32)
                    nc.vector.tensor_reduce(
                        out=s_part,
                        in_=wh,
                        axis=mybir.AxisListType.X,
                        op=mybir.AluOpType.add,
                    )
                    if pt == 0:
                        nc.vector.tensor_copy(out=s_acc, in_=s_part)
                    else:
                        nc.vector.tensor_add(s_acc, s_acc, s_part)

                prev = None
                for t in range(NT):
                    # ---- load hT tile [F=4x128, T] ----
                    ht = hin.tile([128, FC, T], BF16)
                    nc.sync.dma_start(out=ht, in_=h_d[bag, t])

                    # ---- h1T = relu(W1.T @ hT) ----
                    # Tile tracks deps at whole-tile granularity, so the two
                    # halves live in separate tiles: relu/ag-matmuls on half
                    # A overlap the half-B matmuls.
                    h1_ps = [
                        h1psum.tile([128, 2, T], F32, tag="h1ps_a", name="h1ps_a"),
                        h1psum.tile([128, 2, T], F32, tag="h1ps_b", name="h1ps_b"),
                    ]
                    for mc in range(HC):
                        for kc in range(FC):
                            nc.tensor.matmul(
                                h1_ps[mc // 2][:, mc % 2, :],
                                lhsT=w1_sb[:, kc, ts(mc, 128)],
                                rhs=ht[:, kc, :],
                                start=(kc == 0),
                                stop=(kc == FC - 1 and not use_bias),
                            )
                        if use_bias:
                            nc.tensor.matmul(
                                h1_ps[mc // 2][:, mc % 2, :],
                                lhsT=b1_sb[0:1, ts(mc, 128)],
                                rhs=ones_sb[0:1, :],
                                start=False,
                                stop=True,
                            )
                    # previous tile's score path runs here, hidden behind
                    # the h1 matmuls above
                    if prev is not None:
                        score_path(prev)

                    h1_sb = [
                        h1pool.tile([128, 2, T], BF16, tag="h1sb_a", name="h1sb_a"),
                        h1pool.tile([128, 2, T], BF16, tag="h1sb_b", name="h1sb_b"),
                    ]
                    nc.scalar.activation(h1_sb[0], h1_ps[0], mybir.ActivationFunctionType.Relu)
                    nc.scalar.activation(h1_sb[1], h1_ps[1], mybir.ActivationFunctionType.Relu)

                    # ---- aT / gT pre-activations (k-outer: consume h1
                    # chunks as they become ready) ----
                    ag_ps = agpsum.tile([128, HC, T], F32)  # a in 0:2, g in 2:4
                    for kc in range(HC):
                        for half, w_sb_blocks in ((0, wa_sb), (1, wb2_sb)):
                            for mc in range(DC):
                                nc.tensor.matmul(
                                    ag_ps[:, half * DC + mc, :],
                                    lhsT=w_sb_blocks[:, kc, ts(mc, 128)],
                                    rhs=h1_sb[kc // 2][:, kc % 2, :],
                                    start=(kc == 0),
                                    stop=(kc == HC - 1 and not use_bias),
                                )
                    if use_bias:
                        for half in range(2):
                            brow = ba_sb if half == 0 else bb2_sb
                            for mc in range(DC):
                                nc.tensor.matmul(
                                    ag_ps[:, half * DC + mc, :],
                                    lhsT=brow[0:1, ts(mc, 128)],
                                    rhs=ones_sb[0:1, :],
                                    start=False,
                                    stop=True,
                                )
                    a_sb = actp.tile([128, DC, T], BF16)
                    nc.scalar.activation(a_sb, ag_ps[:, 0:DC, :], mybir.ActivationFunctionType.Tanh)
                    g_sb = actp.tile([128, DC, T], BF16)
                    nc.scalar.activation(g_sb, ag_ps[:, DC : 2 * DC, :], mybir.ActivationFunctionType.Tanh)

                    # ---- s' = a + a*g  (= 2*a*sigmoid-gate) ----
                    t1 = actp.tile([128, DC, T], BF16)
                    nc.vector.tensor_mul(t1, a_sb, g_sb)
                    s_sb = actp.tile([128, DC, T], BF16)
                    nc.vector.tensor_add(s_sb, a_sb, t1)

                    prev = (t, ag_ps, s_sb, h1_sb)

                # ---- bag epilogue: flush last tile's score path ----
                score_path(prev)
                nc.sync.dma_start(out=a_out[bag : bag + 1, :], in_=a_row)
                nc.sync.dma_start(out=s_out[bag], in_=s_acc)
    nc.finalize()
    return nc


def kernel(h, W1, b1, Wa, ba, Wb, bb, Wc, bc, Wcls, bcls):
    global LAST_EXEC_NS
    h = np.asarray(h, dtype=np.float32)
    W1 = np.asarray(W1, dtype=np.float32)
    b1 = np.asarray(b1, dtype=np.float32)
    Wa = np.asarray(Wa, dtype=np.float32)
    ba = np.asarray(ba, dtype=np.float32)
    Wb = np.asarray(Wb, dtype=np.float32)
    bb = np.asarray(bb, dtype=np.float32)
    Wc = np.asarray(Wc, dtype=np.float32)
    bc = np.asarray(bc, dtype=np.float32)
    Wcls = np.asarray(Wcls, dtype=np.float32)
    bcls = np.asarray(bcls, dtype=np.float32)

    bf = ml_dtypes.bfloat16
    use_bias = bool(np.any(b1) or np.any(ba) or np.any(bb))

    key = use_bias
    if key not in _CACHE:
        _CACHE[key] = _build(use_bias)
    nc = _CACHE[key]

    # host-side input prep: transpose h to [bag, F, N], fold the
    # sigmoid->tanh rewrite into Wb/bb, replicate Wc/2 across 128 cols
    hT = h.transpose(0, 2, 1).reshape(B, FC, 128, NT, T)
    hT = np.ascontiguousarray(hT.transpose(0, 3, 2, 1, 4)).astype(bf)
    w1_b = W1.astype(bf)
    wa_b = Wa.astype(bf)
    wb2_b = (Wb * 0.5).astype(bf)
    wc2r_b = np.ascontiguousarray(np.repeat(Wc * 0.5, 128, axis=1)).astype(bf)

    in_maps = []
    for c in range(NCORES):
        m = {
            "h": hT[c * BAGS_PER_CORE : (c + 1) * BAGS_PER_CORE],
            "w1": w1_b,
            "wa": wa_b,
            "wb2": wb2_b,
            "wc2r": wc2r_b,
        }
        if use_bias:
            m["b1r"] = b1.reshape(1, H).astype(bf)
            m["bar"] = ba.reshape(1, D).astype(bf)
            m["bb2r"] = (bb * 0.5).reshape(1, D).astype(bf)
        in_maps.append(m)

    res = run_bass_kernel_spmd(
        nc, in_maps, core_ids=list(range(NCORES)), trace=PROFILE
    )
    LAST_EXEC_NS = res.exec_time_ns
    outs = res.results

    A_dev = np.concatenate([o["a_out"] for o in outs], axis=0)  # [16, 8192] f32
    S_dev = np.concatenate([o["s_out"] for o in outs], axis=0)  # [16, 128, HC]
    S_dev = np.ascontiguousarray(S_dev.transpose(0, 2, 1)).reshape(B, H)

    # host epilogue (the "[1,512] all-reduce + classifier" in the hint)
    # softmax weights on device were bf16(exp(A)); reproduce for Z
    w_host = np.exp(A_dev).astype(bf).astype(np.float32)
    Z = w_host.sum(axis=1, keepdims=True)  # [16, 1]
    M = (S_dev / Z).astype(np.float32)  # [16, 512]

    A_raw = (A_dev + bc[0]).reshape(B, 1, N).astype(np.float32)
    M_out = M.reshape(B, 1, H)
    context = M.mean(axis=0, keepdims=True)  # [1, 512]
    logits = (context @ Wcls + bcls).astype(np.float32)  # [1, 2]
    ex = np.exp(logits - logits.max(axis=1, keepdims=True))
    Y_prob = (ex / ex.sum(axis=1, keepdims=True)).astype(np.float32)
    Y_hat = np.argmax(logits, axis=1, keepdims=True).astype(np.int32)
    return (logits, Y_prob, Y_hat, A_raw, M_out)


# revision 16
# speedup vs baseline: 1.1920x; 1.1920x over previous
"""Trainium2 Bass kernel for the CARP3D attention-MIL pooling model.

Model (per bag b of B=16, N=8192 instances, F=512 features):
    h1 = relu(h @ W1 + b1)            [B,N,H]   H=512
    a  = tanh(h1 @ Wa + ba)           [B,N,D]   D=256
    g  = sigmoid(h1 @ Wb + bb)        [B,N,D]
    A  = (a*g) @ Wc + bc              [B,N,1]
    A_sm = softmax(A over N)
    M  = A_sm @ h1                    [B,1,H]
    context = mean_b M; logits = context @ Wcls + bcls

Distribution: data-parallel over bags, 2 bags per core on 8 cores.

Layout strategy: the host pre-transposes h to [bag, F, N] bf16 so the
contraction dim (features) is on SBUF partitions for every matmul and no
on-device transposes are needed.  All compute stays in "feature-major"
space:
    h1T[H,n] = relu(W1.T @ hT)       (lhsT = W1 blocks, rhs = hT)
    aT[D,n]  = tanh(Wa.T @ h1T)
    gT       = tanh(0.5*Wb.T @ h1T)  (sigmoid(x) = 0.5*(1+tanh(x/2)))
    s'       = aT + aT*gT            (= 2*a*g)
    A_rep    = (0.5*Wc replicated 128x).T @ s'   -> [128, n] with every
               partition holding the same score row (free broadcast)
    w        = exp(A_rep)            (no max-subtraction: |A| << 1 here)
    S[H]    += sum_n h1T[H,n]*w[n]   (fused DVE tensor_tensor_reduce)
Outputs per core: raw score rows A (f32) and unnormalized pooled S (f32).
The softmax normalization (Z), bc shift (softmax-invariant), mean over
bags and the [1,512]@[512,2] classifier run on host - microseconds of
work vs the 256MB streaming on device.
"""

import os
import sys

import numpy as np
import ml_dtypes

for _p in ("/opt/trn_rl_repo",):
    if _p not in sys.path:
        sys.path.insert(0, _p)

import concourse.bass as bass
import concourse.mybir as mybir
import concourse.tile as tile
from concourse import bacc
from concourse.bass import ts
from concourse.bass_utils import run_bass_kernel_spmd

BF16 = mybir.dt.bfloat16
F32 = mybir.dt.float32

B, N, F, H, D = 16, 8192, 512, 512, 256
NCORES = 8
BAGS_PER_CORE = B // NCORES          # 2
T = 512                              # instance tile size
NT = N // T                          # 16 tiles per bag
FC = F // 128                        # 4 F-chunks
HC = H // 128                        # 4 H-chunks
DC = D // 128                        # 2 D-chunks

# set by test.py to collect a hardware profile
PROFILE = False
LAST_EXEC_NS = None
_CACHE = {}


def _build(use_bias: bool):
    nc = bacc.Bacc("TRN2", target_bir_lowering=False)

    # h layout: [bag, n_tile, partition, f_chunk, n_in_tile] so each
    # partition's per-tile data is one contiguous 4KB run (fast descriptors)
    h_d = nc.dram_tensor("h", [BAGS_PER_CORE, NT, 128, FC, T], BF16, kind="ExternalInput")
    w1_d = nc.dram_tensor("w1", [F, H], BF16, kind="ExternalInput")
    wa_d = nc.dram_tensor("wa", [H, D], BF16, kind="ExternalInput")
    wb2_d = nc.dram_tensor("wb2", [H, D], BF16, kind="ExternalInput")
    wc2r_d = nc.dram_tensor("wc2r", [D, 128], BF16, kind="ExternalInput")
    if use_bias:
        b1_d = nc.dram_tensor("b1r", [1, H], BF16, kind="ExternalInput")
        ba_d = nc.dram_tensor("bar", [1, D], BF16, kind="ExternalInput")
        bb2_d = nc.dram_tensor("bb2r", [1, D], BF16, kind="ExternalInput")
    a_out = nc.dram_tensor("a_out", [BAGS_PER_CORE, N], BF16, kind="ExternalOutput")
    # partition-major so the DMA writes 16B runs, not 4B scatter
    s_out = nc.dram_tensor("s_out", [BAGS_PER_CORE, 128, HC], F32, kind="ExternalOutput")

    with tile.TileContext(nc) as tc:
        with (
            tc.tile_pool(name="const", bufs=1) as const,
            tc.tile_pool(name="hin", bufs=4) as hin,
            tc.tile_pool(name="h1pool", bufs=2) as h1pool,
            tc.tile_pool(name="acts", bufs=2) as actp,
            tc.tile_pool(name="wexp", bufs=2) as wexp,
            tc.tile_pool(name="scratch", bufs=1) as scratch,
            tc.tile_pool(name="spart", bufs=2) as spartp,
            tc.tile_pool(name="sacc", bufs=2) as saccp,
            tc.tile_pool(name="arow", bufs=2) as arowp,
            tc.tile_pool(name="h1psum", bufs=1, space="PSUM") as h1psum,
            tc.tile_pool(name="agpsum", bufs=1, space="PSUM") as agpsum,
        ):
            # --- weights, loaded once ---
            w1_sb = const.tile([128, FC, H], BF16)
            nc.sync.dma_start(out=w1_sb, in_=w1_d.rearrange("(kc p) h -> p kc h", p=128))
            wa_sb = const.tile([128, HC, D], BF16)
            nc.sync.dma_start(out=wa_sb, in_=wa_d.rearrange("(kc p) d -> p kc d", p=128))
            wb2_sb = const.tile([128, HC, D], BF16)
            nc.sync.dma_start(out=wb2_sb, in_=wb2_d.rearrange("(kc p) d -> p kc d", p=128))
            wc2r_sb = const.tile([128, DC, 128], BF16)
            nc.sync.dma_start(out=wc2r_sb, in_=wc2r_d.rearrange("(kc p) m -> p kc m", p=128))
            if use_bias:
                b1_sb = const.tile([1, H], BF16)
                nc.sync.dma_start(out=b1_sb, in_=b1_d)
                ba_sb = const.tile([1, D], BF16)
                nc.sync.dma_start(out=ba_sb, in_=ba_d)
                bb2_sb = const.tile([1, D], BF16)
                nc.sync.dma_start(out=bb2_sb, in_=bb2_d)
                ones_sb = const.tile([1, T], BF16)
                nc.vector.memset(ones_sb, 1.0)

            for bag in range(BAGS_PER_CORE):
                s_acc = saccp.tile([128, HC], F32)
                a_row = arowp.tile([1, N], BF16)

                # score path of tile t-1, emitted during tile t so exp /
                # a_row / pooling hide behind tile t's h1 matmuls and the
                # ag psum slot is free again before tile t's ag matmuls
                def score_path(prev, s_acc=s_acc, a_row=a_row):
                    pt, p_ag, p_s, p_h1 = prev
                    for kc in range(DC):
                        nc.tensor.matmul(
                            p_ag[:, 0, :],
                            lhsT=wc2r_sb[:, kc, :],
                            rhs=p_s[:, kc, :],
                            start=(kc == 0),
                            stop=(kc == DC - 1),
                        )
                    arep = wexp.tile([128, T], BF16, tag="arep", name="arep")
                    nc.vector.tensor_copy(out=arep, in_=p_ag[:, 0, :])
                    w_sb = wexp.tile([128, T], BF16)
                    nc.scalar.activation(w_sb, arep, mybir.ActivationFunctionType.Exp)
                    nc.vector.tensor_copy(out=a_row[0:1, ts(pt, T)], in_=arep[0:1, :])
                    wh = scratch.tile([128, HC, T], BF16)
                    w_bc2 = bass.AP(
                        tensor=w_sb.tensor,
                        offset=w_sb.offset,
                        ap=[w_sb.ap[0], [0, 2], w_sb.ap[1]],
                    )
                    nc.vector.tensor_mul(wh[:, 0:2, :], p_h1[0], w_bc2)
                    nc.vector.tensor_mul(wh[:, 2:4, :], p_h1[1], w_bc2)
                    s_part = spartp.tile([128, HC], F32)
                    nc.vector.tensor_reduce(
                        out=s_part,
                        in_=wh,
                        axis=mybir.AxisListType.X,
                        op=mybir.AluOpType.add,
                    )
                    if pt == 0:
                        nc.vector.tensor_copy(out=s_acc, in_=s_part)
                    else:
                        nc.vector.tensor_add(s_acc, s_acc, s_part)

                prev = None
                for t in range(NT):
                    # ---- load hT tile [F=4x128, T] ----
                    ht = hin.tile([128, FC, T], BF16)
                    nc.sync.dma_start(out=ht, in_=h_d[bag, t])

                    # ---- h1T = relu(W1.T @ hT) ----
                    # Tile tracks deps at whole-tile granularity, so the two
                    # halves live in separate tiles: relu/ag-matmuls on half
                    # A overlap the half-B matmuls.
                    h1_ps = [
                        h1psum.tile([128, 2, T], F32, tag="h1ps_a", name="h1ps_a"),
                        h1psum.tile([128, 2, T], F32, tag="h1ps_b", name="h1ps_b"),
                    ]
                    for mc in range(HC):
                        for kc in range(FC):
                            nc.tensor.matmul(
                                h1_ps[mc // 2][:, mc % 2, :],
                                lhsT=w1_sb[:, kc, ts(mc, 128)],
                                rhs=ht[:, kc, :],
                                start=(kc == 0),
                                stop=(kc == FC - 1 and not use_bias),
                            )
                        if use_bias:
                            nc.tensor.matmul(
                                h1_ps[mc // 2][:, mc % 2, :],
                                lhsT=b1_sb[0:1, ts(mc, 128)],
                                rhs=ones_sb[0:1, :],
                                start=False,
                                stop=True,
                            )
                    # previous tile's score path runs here, hidden behind
                    # the h1 matmuls above
                    if prev is not None:
                        score_path(prev)

                    h1_sb = [
                        h1pool.tile([128, 2, T], BF16, tag="h1sb_a", name="h1sb_a"),
                        h1pool.tile([128, 2, T], BF16, tag="h1sb_b", name="h1sb_b"),
                    ]
                    nc.scalar.activation(h1_sb[0], h1_ps[0], mybir.ActivationFunctionType.Relu)
                    nc.scalar.activation(h1_sb[1], h1_ps[1], mybir.ActivationFunctionType.Relu)

                    # ---- aT / gT pre-activations (k-outer: consume h1
                    # chunks as they become ready) ----
                    # bf16 psum: 2 banks per tile so two tiles fit in
                    # flight; start=True clears has_written for the WHOLE
                    # bank, so only the bank's first chunk may set it
                    ag_ps = agpsum.tile([128, HC, T], F32)  # a in 0:2, g in 2:4
                    for kc in range(HC):
                        for half, w_sb_blocks in ((0, wa_sb), (1, wb2_sb)):
                            for mc in range(DC):
                                nc.tensor.matmul(
                                    ag_ps[:, half * DC + mc, :],
                                    lhsT=w_sb_blocks[:, kc, ts(mc, 128)],
                                    rhs=h1_sb[kc // 2][:, kc % 2, :],
                                    start=(kc == 0),
                                    stop=(kc == HC - 1 and not use_bias),
                                )
                    if use_bias:
                        for half in range(2):
                            brow = ba_sb if half == 0 else bb2_sb
                            for mc in range(DC):
                                nc.tensor.matmul(
                                    ag_ps[:, half * DC + mc, :],
                                    lhsT=brow[0:1, ts(mc, 128)],
                                    rhs=ones_sb[0:1, :],
                                    start=False,
                                    stop=True,
                                )
                    ag_sb = actp.tile([128, HC, T], BF16)
                    nc.scalar.activation(ag_sb, ag_ps, mybir.ActivationFunctionType.Tanh)

                    # ---- s' = a*(1+g)  (= 2*a*sigmoid-gate) ----
                    gp1 = actp.tile([128, DC, T], BF16)
                    nc.vector.tensor_scalar_add(gp1, ag_sb[:, DC : 2 * DC, :], 1.0)
                    s_sb = actp.tile([128, DC, T], BF16)
                    nc.vector.tensor_mul(s_sb, ag_sb[:, 0:DC, :], gp1)

                    prev = (t, ag_ps, s_sb, h1_sb)

                # ---- bag epilogue: flush last tile's score path ----
                score_path(prev)
                nc.sync.dma_start(out=a_out[bag : bag + 1, :], in_=a_row)
                nc.sync.dma_start(out=s_out[bag], in_=s_acc)
    nc.finalize()
    return nc


def kernel(h, W1, b1, Wa, ba, Wb, bb, Wc, bc, Wcls, bcls):
    global LAST_EXEC_NS
    h = np.asarray(h, dtype=np.float32)
    W1 = np.asarray(W1, dtype=np.float32)
    b1 = np.asarray(b1, dtype=np.float32)
    Wa = np.asarray(Wa, dtype=np.float32)
    ba = np.asarray(ba, dtype=np.float32)
    Wb = np.asarray(Wb, dtype=np.float32)
    bb = np.asarray(bb, dtype=np.float32)
    Wc = np.asarray(Wc, dtype=np.float32)
    bc = np.asarray(bc, dtype=np.float32)
    Wcls = np.asarray(Wcls, dtype=np.float32)
    bcls = np.asarray(bcls, dtype=np.float32)

    bf = ml_dtypes.bfloat16
    use_bias = bool(np.any(b1) or np.any(ba) or np.any(bb))

    key = use_bias
    if key not in _CACHE:
        _CACHE[key] = _build(use_bias)
    nc = _CACHE[key]

    # host-side input prep: transpose h to [bag, F, N], fold the
    # sigmoid->tanh rewrite into Wb/bb, replicate Wc/2 across 128 cols
    hT = h.transpose(0, 2, 1).reshape(B, FC, 128, NT, T)
    hT = np.ascontiguousarray(hT.transpose(0, 3, 2, 1, 4)).astype(bf)
    w1_b = W1.astype(bf)
    wa_b = Wa.astype(bf)
    wb2_b = (Wb * 0.5).astype(bf)
    wc2r_b = np.ascontiguousarray(np.repeat(Wc * 0.5, 128, axis=1)).astype(bf)

    in_maps = []
    for c in range(NCORES):
        m = {
            "h": hT[c * BAGS_PER_CORE : (c + 1) * BAGS_PER_CORE],
            "w1": w1_b,
            "wa": wa_b,
            "wb2": wb2_b,
            "wc2r": wc2r_b,
        }
        if use_bias:
            m["b1r"] = b1.reshape(1, H).astype(bf)
            m["bar"] = ba.reshape(1, D).astype(bf)
            m["bb2r"] = (bb * 0.5).reshape(1, D).astype(bf)
        in_maps.append(m)

    res = run_bass_kernel_spmd(
        nc, in_maps, core_ids=list(range(NCORES)), trace=PROFILE
    )
    LAST_EXEC_NS = res.exec_time_ns
    outs = res.results

    A_dev = np.concatenate([o["a_out"] for o in outs], axis=0).astype(np.float32)
    S_dev = np.concatenate([o["s_out"] for o in outs], axis=0)  # [16, 128, HC]
    S_dev = np.ascontiguousarray(S_dev.transpose(0, 2, 1)).reshape(B, H)

    # host epilogue (the "[1,512] all-reduce + classifier" in the hint)
    # softmax weights on device were bf16(exp(A)); reproduce for Z
    w_host = np.exp(A_dev).astype(bf).astype(np.float32)
    Z = w_host.sum(axis=1, keepdims=True)  # [16, 1]
    M = (S_dev / Z).astype(np.float32)  # [16, 512]

    A_raw = (A_dev + bc[0]).reshape(B, 1, N).astype(np.float32)
    M_out = M.reshape(B, 1, H)
    context = M.mean(axis=0, keepdims=True)  # [1, 512]
    logits = (context @ Wcls + bcls).astype(np.float32)  # [1, 2]
    ex = np.exp(logits - logits.max(axis=1, keepdims=True))
    Y_prob = (ex / ex.sum(axis=1, keepdims=True)).astype(np.float32)
    Y_hat = np.argmax(logits, axis=1, keepdims=True).astype(np.int32)
    return (logits, Y_prob, Y_hat, A_raw, M_out)


# revision 17
# speedup vs baseline: 1.3466x; 1.1298x over previous
"""Trainium2 Bass kernel for the CARP3D attention-MIL pooling model.

Model (per bag b of B=16, N=8192 instances, F=512 features):
    h1 = relu(h @ W1 + b1)            [B,N,H]   H=512
    a  = tanh(h1 @ Wa + ba)           [B,N,D]   D=256
    g  = sigmoid(h1 @ Wb + bb)        [B,N,D]
    A  = (a*g) @ Wc + bc              [B,N,1]
    A_sm = softmax(A over N)
    M  = A_sm @ h1                    [B,1,H]
    context = mean_b M; logits = context @ Wcls + bcls

Distribution: data-parallel over bags, 2 bags per core on 8 cores.

Layout strategy: the host pre-transposes h to [bag, F, N] bf16 so the
contraction dim (features) is on SBUF partitions for every matmul and no
on-device transposes are needed.  All compute stays in "feature-major"
space:
    h1T[H,n] = relu(W1.T @ hT)       (lhsT = W1 blocks, rhs = hT)
    aT[D,n]  = tanh(Wa.T @ h1T)
    gT       = tanh(0.5*Wb.T @ h1T)  (sigmoid(x) = 0.5*(1+tanh(x/2)))
    s'       = aT + aT*gT            (= 2*a*g)
    A_rep    = (0.5*Wc replicated 128x).T @ s'   -> [128, n] with every
               partition holding the same score row (free broadcast)
    w        = exp(A_rep)            (no max-subtraction: |A| << 1 here)
    S[H]    += sum_n h1T[H,n]*w[n]   (fused DVE tensor_tensor_reduce)
Outputs per core: raw score rows A (f32) and unnormalized pooled S (f32).
The softmax normalization (Z), bc shift (softmax-invariant), mean over
bags and the [1,512]@[512,2] classifier run on host - microseconds of
work vs the 256MB streaming on device.
"""

import os
import sys

import numpy as np
import ml_dtypes

for _p in ("/opt/trn_rl_repo",):
    if _p not in sys.path:
        sys.path.insert(0, _p)

import concourse.bass as bass
import concourse.mybir as mybir
import concourse.tile as tile
from concourse import bacc
from concourse.bass import ts
from concourse.bass_utils import run_bass_kernel_spmd

BF16 = mybir.dt.bfloat16
F32 = mybir.dt.float32

B, N, F, H, D = 16, 8192, 512, 512, 256
NCORES = 8
BAGS_PER_CORE = B // NCORES          # 2
T = 512                              # instance tile size
NT = N // T                          # 16 tiles per bag
FC = F // 128                        # 4 F-chunks
HC = H // 128                        # 4 H-chunks
DC = D // 128                        # 2 D-chunks

# set by test.py to collect a hardware profile
PROFILE = False
LAST_EXEC_NS = None
_CACHE = {}


def _build(use_bias: bool):
    nc = bacc.Bacc("TRN2", target_bir_lowering=False)

    # h layout: [bag, n_tile, partition, f_chunk, n_in_tile] so each
    # partition's per-tile data is one contiguous 4KB run (fast descriptors)
    h_d = nc.dram_tensor("h", [BAGS_PER_CORE, NT, 128, FC, T], BF16, kind="ExternalInput")
    w1_d = nc.dram_tensor("w1", [F, H], BF16, kind="ExternalInput")
    wa_d = nc.dram_tensor("wa", [H, D], BF16, kind="ExternalInput")
    wb2_d = nc.dram_tensor("wb2", [H, D], BF16, kind="ExternalInput")
    wc2r_d = nc.dram_tensor("wc2r", [D, 128], BF16, kind="ExternalInput")
    if use_bias:
        b1_d = nc.dram_tensor("b1r", [1, H], BF16, kind="ExternalInput")
        ba_d = nc.dram_tensor("bar", [1, D], BF16, kind="ExternalInput")
        bb2_d = nc.dram_tensor("bb2r", [1, D], BF16, kind="ExternalInput")
    a_out = nc.dram_tensor("a_out", [BAGS_PER_CORE, N], BF16, kind="ExternalOutput")
    # partition-major so the DMA writes 16B runs, not 4B scatter
    s_out = nc.dram_tensor("s_out", [BAGS_PER_CORE, 128, HC], F32, kind="ExternalOutput")

    with tile.TileContext(nc) as tc:
        with (
            tc.tile_pool(name="const", bufs=1) as const,
            tc.tile_pool(name="hin", bufs=4) as hin,
            tc.tile_pool(name="h1pool", bufs=2) as h1pool,
            tc.tile_pool(name="acts", bufs=2) as actp,
            tc.tile_pool(name="wexp", bufs=2) as wexp,
            tc.tile_pool(name="scratch", bufs=1) as scratch,
            tc.tile_pool(name="spart", bufs=2) as spartp,
            tc.tile_pool(name="sacc", bufs=2) as saccp,
            tc.tile_pool(name="arow", bufs=2) as arowp,
            tc.tile_pool(name="h1psum", bufs=1, space="PSUM") as h1psum,
            tc.tile_pool(name="agpsum", bufs=1, space="PSUM") as agpsum,
        ):
            # --- weights, loaded once (w1 + first h tile first: they gate
            # the first matmul; the rest follows) ---
            w1_sb = const.tile([128, FC, H], BF16)
            nc.sync.dma_start(out=w1_sb, in_=w1_d.rearrange("(kc p) h -> p kc h", p=128))
            ht0 = hin.tile([128, FC, T], BF16, tag="ht", name="ht")
            nc.sync.dma_start(out=ht0, in_=h_d[0, 0])
            wa_sb = const.tile([128, HC, D], BF16)
            nc.sync.dma_start(out=wa_sb, in_=wa_d.rearrange("(kc p) d -> p kc d", p=128))
            wb2_sb = const.tile([128, HC, D], BF16)
            nc.sync.dma_start(out=wb2_sb, in_=wb2_d.rearrange("(kc p) d -> p kc d", p=128))
            wc2r_sb = const.tile([128, DC, 128], BF16)
            nc.sync.dma_start(out=wc2r_sb, in_=wc2r_d.rearrange("(kc p) m -> p kc m", p=128))
            if use_bias:
                b1_sb = const.tile([1, H], BF16)
                nc.sync.dma_start(out=b1_sb, in_=b1_d)
                ba_sb = const.tile([1, D], BF16)
                nc.sync.dma_start(out=ba_sb, in_=ba_d)
                bb2_sb = const.tile([1, D], BF16)
                nc.sync.dma_start(out=bb2_sb, in_=bb2_d)
                ones_sb = const.tile([1, T], BF16)
                nc.vector.memset(ones_sb, 1.0)

            for bag in range(BAGS_PER_CORE):
                s_acc = saccp.tile([128, HC], F32)
                a_row = arowp.tile([1, N], BF16)

                # score path of tile t-1, emitted during tile t so exp /
                # a_row / pooling hide behind tile t's h1 matmuls and the
                # ag psum slot is free again before tile t's ag matmuls
                def score_path(prev, s_acc=s_acc, a_row=a_row):
                    pt, p_ag, p_s, p_h1 = prev  # p_ag = that tile's a_ps
                    for kc in range(DC):
                        nc.tensor.matmul(
                            p_ag[:, 0, :],
                            lhsT=wc2r_sb[:, kc, :],
                            rhs=p_s[:, kc, :],
                            start=(kc == 0),
                            stop=(kc == DC - 1),
                        )
                    arep = wexp.tile([128, T], BF16, tag="arep", name="arep")
                    nc.vector.tensor_copy(out=arep, in_=p_ag[:, 0, :])
                    w_sb = wexp.tile([128, T], BF16)
                    nc.scalar.activation(w_sb, arep, mybir.ActivationFunctionType.Exp)
                    nc.vector.tensor_copy(out=a_row[0:1, ts(pt, T)], in_=arep[0:1, :])
                    wh = scratch.tile([128, HC, T], BF16)
                    w_bc2 = bass.AP(
                        tensor=w_sb.tensor,
                        offset=w_sb.offset,
                        ap=[w_sb.ap[0], [0, 2], w_sb.ap[1]],
                    )
                    nc.vector.tensor_mul(wh[:, 0:2, :], p_h1[0], w_bc2)
                    nc.vector.tensor_mul(wh[:, 2:4, :], p_h1[1], w_bc2)
                    s_part = spartp.tile([128, HC], F32)
                    nc.vector.tensor_reduce(
                        out=s_part,
                        in_=wh,
                        axis=mybir.AxisListType.X,
                        op=mybir.AluOpType.add,
                    )
                    if pt == 0:
                        nc.vector.tensor_copy(out=s_acc, in_=s_part)
                    else:
                        nc.vector.tensor_add(s_acc, s_acc, s_part)

                prev = None
                for t in range(NT):
                    # ---- load hT tile [F=4x128, T] ----
                    if bag == 0 and t == 0:
                        ht = ht0
                    else:
                        ht = hin.tile([128, FC, T], BF16, tag="ht", name="ht")
                        nc.sync.dma_start(out=ht, in_=h_d[bag, t])

                    # ---- h1T = relu(W1.T @ hT) ----
                    # Tile tracks deps at whole-tile granularity, so the two
                    # halves live in separate tiles: relu/ag-matmuls on half
                    # A overlap the half-B matmuls.
                    h1_ps = [
                        h1psum.tile([128, 2, T], F32, tag="h1ps_a", name="h1ps_a"),
                        h1psum.tile([128, 2, T], F32, tag="h1ps_b", name="h1ps_b"),
                    ]
                    for mc in range(HC):
                        for kc in range(FC):
                            nc.tensor.matmul(
                                h1_ps[mc // 2][:, mc % 2, :],
                                lhsT=w1_sb[:, kc, ts(mc, 128)],
                                rhs=ht[:, kc, :],
                                start=(kc == 0),
                                stop=(kc == FC - 1 and not use_bias),
                            )
                        if use_bias:
                            nc.tensor.matmul(
                                h1_ps[mc // 2][:, mc % 2, :],
                                lhsT=b1_sb[0:1, ts(mc, 128)],
                                rhs=ones_sb[0:1, :],
                                start=False,
                                stop=True,
                            )
                    # previous tile's score path runs here, hidden behind
                    # the h1 matmuls above
                    if prev is not None:
                        score_path(prev)

                    h1_sb = [
                        h1pool.tile([128, 2, T], BF16, tag="h1sb_a", name="h1sb_a"),
                        h1pool.tile([128, 2, T], BF16, tag="h1sb_b", name="h1sb_b"),
                    ]
                    nc.scalar.activation(h1_sb[0], h1_ps[0], mybir.ActivationFunctionType.Relu)
                    nc.scalar.activation(h1_sb[1], h1_ps[1], mybir.ActivationFunctionType.Relu)

                    # ---- aT / gT pre-activations (k-outer: consume h1
                    # chunks as they become ready) ----
                    # a and g in separate psum tiles, all a-matmuls first:
                    # tanh(a) runs while the g-matmuls stream, shortening the
                    # per-tile critical chain
                    a_ps = agpsum.tile([128, DC, T], F32, tag="a_ps", name="a_ps")
                    g_ps = agpsum.tile([128, DC, T], F32, tag="g_ps", name="g_ps")
                    for ps, w_sb_blocks, brow_name in ((a_ps, wa_sb, "ba"), (g_ps, wb2_sb, "bb2")):
                        for kc in range(HC):
                            for mc in range(DC):
                                nc.tensor.matmul(
                                    ps[:, mc, :],
                                    lhsT=w_sb_blocks[:, kc, ts(mc, 128)],
                                    rhs=h1_sb[kc // 2][:, kc % 2, :],
                                    start=(kc == 0),
                                    stop=(kc == HC - 1 and not use_bias),
                                )
                        if use_bias:
                            brow = ba_sb if ps is a_ps else bb2_sb
                            for mc in range(DC):
                                nc.tensor.matmul(
                                    ps[:, mc, :],
                                    lhsT=brow[0:1, ts(mc, 128)],
                                    rhs=ones_sb[0:1, :],
                                    start=False,
                                    stop=True,
                                )
                    a_sb = actp.tile([128, DC, T], BF16)
                    nc.scalar.activation(a_sb, a_ps, mybir.ActivationFunctionType.Tanh)
                    g_sb = actp.tile([128, DC, T], BF16)
                    nc.scalar.activation(g_sb, g_ps, mybir.ActivationFunctionType.Tanh)

                    # ---- s' = a*(1+g)  (= 2*a*sigmoid-gate) ----
                    gp1 = actp.tile([128, DC, T], BF16)
                    nc.vector.tensor_scalar_add(gp1, g_sb, 1.0)
                    s_sb = actp.tile([128, DC, T], BF16)
                    nc.vector.tensor_mul(s_sb, a_sb, gp1)

                    prev = (t, a_ps, s_sb, h1_sb)

                # ---- bag epilogue: flush last tile's score path ----
                score_path(prev)
                nc.sync.dma_start(out=a_out[bag : bag + 1, :], in_=a_row)
                nc.sync.dma_start(out=s_out[bag], in_=s_acc)
    nc.finalize()
    return nc


def kernel(h, W1, b1, Wa, ba, Wb, bb, Wc, bc, Wcls, bcls):
    global LAST_EXEC_NS
    h = np.asarray(h, dtype=np.float32)
    W1 = np.asarray(W1, dtype=np.float32)
    b1 = np.asarray(b1, dtype=np.float32)
    Wa = np.asarray(Wa, dtype=np.float32)
    ba = np.asarray(ba, dtype=np.float32)
    Wb = np.asarray(Wb, dtype=np.float32)
    bb = np.asarray(bb, dtype=np.float32)
    Wc = np.asarray(Wc, dtype=np.float32)
    bc = np.asarray(bc, dtype=np.float32)
    Wcls = np.asarray(Wcls, dtype=np.float32)
    bcls = np.asarray(bcls, dtype=np.float32)

    bf = ml_dtypes.bfloat16
    use_bias = bool(np.any(b1) or np.any(ba) or np.any(bb))

    key = use_bias
    if key not in _CACHE:
        _CACHE[key] = _build(use_bias)
    nc = _CACHE[key]

    # host-side input prep: transpose h to [bag, F, N], fold the
    # sigmoid->tanh rewrite into Wb/bb, replicate Wc/2 across 128 cols
    hT = h.transpose(0, 2, 1).reshape(B, FC, 128, NT, T)
    hT = np.ascontiguousarray(hT.transpose(0, 3, 2, 1, 4)).astype(bf)
    w1_b = W1.astype(bf)
    wa_b = Wa.astype(bf)
    wb2_b = (Wb * 0.5).astype(bf)
    wc2r_b = np.ascontiguousarray(np.repeat(Wc * 0.5, 128, axis=1)).astype(bf)

    in_maps = []
    for c in range(NCORES):
        m = {
            "h": hT[c * BAGS_PER_CORE : (c + 1) * BAGS_PER_CORE],
            "w1": w1_b,
            "wa": wa_b,
            "wb2": wb2_b,
            "wc2r": wc2r_b,
        }
        if use_bias:
            m["b1r"] = b1.reshape(1, H).astype(bf)
            m["bar"] = ba.reshape(1, D).astype(bf)
            m["bb2r"] = (bb * 0.5).reshape(1, D).astype(bf)
        in_maps.append(m)

    res = run_bass_kernel_spmd(
        nc, in_maps, core_ids=list(range(NCORES)), trace=PROFILE
    )
    LAST_EXEC_NS = res.exec_time_ns
    outs = res.results

    A_dev = np.concatenate([o["a_out"] for o in outs], axis=0).astype(np.float32)
    S_dev = np.concatenate([o["s_out"] for o in outs], axis=0)  # [16, 128, HC]
    S_dev = np.ascontiguousarray(S_dev.transpose(0, 2, 1)).reshape(B, H)

    # host epilogue (the "[1,512] all-reduce + classifier" in the hint)
    # softmax weights on device were bf16(exp(A)); reproduce for Z
    w_host = np.exp(A_dev).astype(bf).astype(np.float32)
    Z = w_host.sum(axis=1, keepdims=True)  # [16, 1]
    M = (S_dev / Z).astype(np.float32)  # [16, 512]

    A_raw = (A_dev + bc[0]).reshape(B, 1, N).astype(np.float32)
    M_out = M.reshape(B, 1, H)
    context = M.mean(axis=0, keepdims=True)  # [1, 512]
    logits = (context @ Wcls + bcls).astype(np.float32)  # [1, 2]
    ex = np.exp(logits - logits.max(axis=1, keepdims=True))
    Y_prob = (ex / ex.sum(axis=1, keepdims=True)).astype(np.float32)
    Y_hat = np.argmax(logits, axis=1, keepdims=True).astype(np.int32)
    return (logits, Y_prob, Y_hat, A_raw, M_out)


# revision 19
# speedup vs baseline: 1.3561x; 1.0071x over previous
"""Trainium2 Bass kernel for the CARP3D attention-MIL pooling model.

Model (per bag b of B=16, N=8192 instances, F=512 features):
    h1 = relu(h @ W1 + b1)            [B,N,H]   H=512
    a  = tanh(h1 @ Wa + ba)           [B,N,D]   D=256
    g  = sigmoid(h1 @ Wb + bb)        [B,N,D]
    A  = (a*g) @ Wc + bc              [B,N,1]
    A_sm = softmax(A over N)
    M  = A_sm @ h1                    [B,1,H]
    context = mean_b M; logits = context @ Wcls + bcls

Distribution: data-parallel over bags, 2 bags per core on 8 cores.

Layout strategy: the host pre-transposes h to [bag, F, N] bf16 so the
contraction dim (features) is on SBUF partitions for every matmul and no
on-device transposes are needed.  All compute stays in "feature-major"
space:
    h1T[H,n] = relu(W1.T @ hT)       (lhsT = W1 blocks, rhs = hT)
    aT[D,n]  = tanh(Wa.T @ h1T)
    gT       = tanh(0.5*Wb.T @ h1T)  (sigmoid(x) = 0.5*(1+tanh(x/2)))
    s'       = aT + aT*gT            (= 2*a*g)
    A_rep    = (0.5*Wc replicated 128x).T @ s'   -> [128, n] with every
               partition holding the same score row (free broadcast)
    w        = exp(A_rep)            (no max-subtraction: |A| << 1 here)
    S[H]    += sum_n h1T[H,n]*w[n]   (DVE mul + free-dim reduce)
The per-tile score chain (tanh -> s' -> A -> exp) is software-pipelined
one tile behind the matmul stream so TensorE runs gap-free.
Outputs per core: raw score rows A (bf16) and unnormalized pooled S (f32).
The softmax normalization (Z), bc shift (softmax-invariant), mean over
bags and the [1,512]@[512,2] classifier run on host - microseconds of
work vs the 256MB streaming on device.
"""

import os
import sys

import numpy as np
import ml_dtypes

for _p in ("/opt/trn_rl_repo",):
    if _p not in sys.path:
        sys.path.insert(0, _p)

import concourse.bass as bass
import concourse.mybir as mybir
import concourse.tile as tile
from concourse import bacc
from concourse.bass import ts
from concourse.bass_utils import run_bass_kernel_spmd

BF16 = mybir.dt.bfloat16
F32 = mybir.dt.float32

B, N, F, H, D = 16, 8192, 512, 512, 256
NCORES = 8
BAGS_PER_CORE = B // NCORES          # 2
T = 512                              # instance tile size
NT = N // T                          # 16 tiles per bag
FC = F // 128                        # 4 F-chunks
HC = H // 128                        # 4 H-chunks
DC = D // 128                        # 2 D-chunks

# set by test.py to collect a hardware profile
PROFILE = False
LAST_EXEC_NS = None
_CACHE = {}


def _build(use_bias: bool):
    nc = bacc.Bacc("TRN2", target_bir_lowering=False)

    # h layout: [bag, n_tile, partition, f_chunk, n_in_tile] so each
    # partition's per-tile data is one contiguous 4KB run (fast descriptors)
    h_d = nc.dram_tensor("h", [BAGS_PER_CORE, NT, 128, FC, T], BF16, kind="ExternalInput")
    w1_d = nc.dram_tensor("w1", [F, H], BF16, kind="ExternalInput")
    wa_d = nc.dram_tensor("wa", [H, D], BF16, kind="ExternalInput")
    wb2_d = nc.dram_tensor("wb2", [H, D], BF16, kind="ExternalInput")
    wc2r_d = nc.dram_tensor("wc2r", [D, 128], BF16, kind="ExternalInput")
    if use_bias:
        b1_d = nc.dram_tensor("b1r", [1, H], BF16, kind="ExternalInput")
        ba_d = nc.dram_tensor("bar", [1, D], BF16, kind="ExternalInput")
        bb2_d = nc.dram_tensor("bb2r", [1, D], BF16, kind="ExternalInput")
    a_out = nc.dram_tensor("a_out", [BAGS_PER_CORE, N], BF16, kind="ExternalOutput")
    # partition-major so the DMA writes 16B runs, not 4B scatter
    s_out = nc.dram_tensor("s_out", [BAGS_PER_CORE, 128, HC], F32, kind="ExternalOutput")

    with tile.TileContext(nc) as tc:
        with (
            tc.tile_pool(name="const", bufs=1) as const,
            tc.tile_pool(name="hin", bufs=4) as hin,
            tc.tile_pool(name="h1pool", bufs=2) as h1pool,
            tc.tile_pool(name="acts", bufs=2) as actp,
            tc.tile_pool(name="wexp", bufs=2) as wexp,
            tc.tile_pool(name="scratch", bufs=1) as scratch,
            tc.tile_pool(name="spart", bufs=2) as spartp,
            tc.tile_pool(name="sacc", bufs=2) as saccp,
            tc.tile_pool(name="arow", bufs=2) as arowp,
            tc.tile_pool(name="h1psum", bufs=1, space="PSUM") as h1psum,
            tc.tile_pool(name="agpsum", bufs=1, space="PSUM") as agpsum,
        ):
            # --- weights, loaded once (w1 + first h tile first: they gate
            # the first matmul; the rest follows) ---
            w1_sb = const.tile([128, FC, H], BF16)
            ht0 = hin.tile([128, FC, T], BF16, tag="ht", name="ht")
            w1_r = w1_d.rearrange("(kc p) h -> p kc h", p=128)
            for kc in range(FC):
                nc.sync.dma_start(out=w1_sb[:, kc, :], in_=w1_r[:, kc, :])
                nc.sync.dma_start(out=ht0[:, kc, :], in_=h_d[0, 0, :, kc, :])
            wa_sb = const.tile([128, HC, D], BF16)
            nc.sync.dma_start(out=wa_sb, in_=wa_d.rearrange("(kc p) d -> p kc d", p=128))
            wb2_sb = const.tile([128, HC, D], BF16)
            nc.sync.dma_start(out=wb2_sb, in_=wb2_d.rearrange("(kc p) d -> p kc d", p=128))
            wc2r_sb = const.tile([128, DC, 128], BF16)
            nc.sync.dma_start(out=wc2r_sb, in_=wc2r_d.rearrange("(kc p) m -> p kc m", p=128))
            if use_bias:
                b1_sb = const.tile([1, H], BF16)
                nc.sync.dma_start(out=b1_sb, in_=b1_d)
                ba_sb = const.tile([1, D], BF16)
                nc.sync.dma_start(out=ba_sb, in_=ba_d)
                bb2_sb = const.tile([1, D], BF16)
                nc.sync.dma_start(out=bb2_sb, in_=bb2_d)
                ones_sb = const.tile([1, T], BF16)
                nc.vector.memset(ones_sb, 1.0)

            s_accs = [saccp.tile([128, HC], F32, name=f"s_acc{b}") for b in range(BAGS_PER_CORE)]
            a_rows = [arowp.tile([1, N], BF16, name=f"a_row{b}") for b in range(BAGS_PER_CORE)]

            # score path of tile t-1, emitted during tile t so exp / a_row /
            # pooling hide behind tile t's h1 matmuls; carried across the
            # bag boundary so bags never serialize.  Emits the bag's output
            # DMAs when its last tile retires.
            if True:
                def score_path(prev):
                    pb, pt, p_ag, p_s, p_h1 = prev  # p_ag = that tile's a_ps
                    s_acc = s_accs[pb]
                    a_row = a_rows[pb]
                    for kc in range(DC):
                        nc.tensor.matmul(
                            p_ag[:, 0, :],
                            lhsT=wc2r_sb[:, kc, :],
                            rhs=p_s[:, kc, :],
                            start=(kc == 0),
                            stop=(kc == DC - 1),
                        )
                    arep = wexp.tile([128, T], BF16, tag="arep", name="arep")
                    nc.vector.tensor_copy(out=arep, in_=p_ag[:, 0, :])
                    w_sb = wexp.tile([128, T], BF16)
                    nc.scalar.activation(w_sb, arep, mybir.ActivationFunctionType.Exp)
                    nc.vector.tensor_copy(out=a_row[0:1, ts(pt, T)], in_=arep[0:1, :])
                    wh = scratch.tile([128, HC, T], BF16)
                    w_bc2 = bass.AP(
                        tensor=w_sb.tensor,
                        offset=w_sb.offset,
                        ap=[w_sb.ap[0], [0, 2], w_sb.ap[1]],
                    )
                    nc.vector.tensor_mul(wh[:, 0:2, :], p_h1[0], w_bc2)
                    nc.vector.tensor_mul(wh[:, 2:4, :], p_h1[1], w_bc2)
                    s_part = spartp.tile([128, HC], F32)
                    nc.vector.tensor_reduce(
                        out=s_part,
                        in_=wh,
                        axis=mybir.AxisListType.X,
                        op=mybir.AluOpType.add,
                    )
                    if pt == 0:
                        nc.vector.tensor_copy(out=s_acc, in_=s_part)
                    else:
                        nc.vector.tensor_add(s_acc, s_acc, s_part)
                    if pt == NT - 1:
                        nc.sync.dma_start(out=a_out[pb : pb + 1, :], in_=a_row)
                        nc.sync.dma_start(out=s_out[pb], in_=s_acc)

            prev = None
            for bag in range(BAGS_PER_CORE):
                for t in range(NT):
                    # ---- load hT tile [F=4x128, T] ----
                    if bag == 0 and t == 0:
                        ht = ht0
                    else:
                        ht = hin.tile([128, FC, T], BF16, tag="ht", name="ht")
                        nc.sync.dma_start(out=ht, in_=h_d[bag, t])

                    # ---- h1T = relu(W1.T @ hT) ----
                    # Tile tracks deps at whole-tile granularity, so the two
                    # halves live in separate tiles: relu/ag-matmuls on half
                    # A overlap the half-B matmuls.
                    h1_ps = [
                        h1psum.tile([128, 2, T], F32, tag="h1ps_a", name="h1ps_a"),
                        h1psum.tile([128, 2, T], F32, tag="h1ps_b", name="h1ps_b"),
                    ]
                    for mc in range(HC):
                        for kc in range(FC):
                            nc.tensor.matmul(
                                h1_ps[mc // 2][:, mc % 2, :],
                                lhsT=w1_sb[:, kc, ts(mc, 128)],
                                rhs=ht[:, kc, :],
                                start=(kc == 0),
                                stop=(kc == FC - 1 and not use_bias),
                            )
                        if use_bias:
                            nc.tensor.matmul(
                                h1_ps[mc // 2][:, mc % 2, :],
                                lhsT=b1_sb[0:1, ts(mc, 128)],
                                rhs=ones_sb[0:1, :],
                                start=False,
                                stop=True,
                            )
                    # previous tile's score path runs here, hidden behind
                    # the h1 matmuls above
                    if prev is not None:
                        score_path(prev)

                    h1_sb = [
                        h1pool.tile([128, 2, T], BF16, tag="h1sb_a", name="h1sb_a"),
                        h1pool.tile([128, 2, T], BF16, tag="h1sb_b", name="h1sb_b"),
                    ]
                    nc.scalar.activation(h1_sb[0], h1_ps[0], mybir.ActivationFunctionType.Relu)
                    nc.scalar.activation(h1_sb[1], h1_ps[1], mybir.ActivationFunctionType.Relu)

                    # ---- aT / gT pre-activations (k-outer: consume h1
                    # chunks as they become ready) ----
                    # a and g in separate psum tiles, all a-matmuls first:
                    # tanh(a) runs while the g-matmuls stream, shortening the
                    # per-tile critical chain
                    a_ps = agpsum.tile([128, DC, T], F32, tag="a_ps", name="a_ps")
                    g_ps = agpsum.tile([128, DC, T], F32, tag="g_ps", name="g_ps")
                    for ps, w_sb_blocks, brow_name in ((a_ps, wa_sb, "ba"), (g_ps, wb2_sb, "bb2")):
                        for kc in range(HC):
                            for mc in range(DC):
                                nc.tensor.matmul(
                                    ps[:, mc, :],
                                    lhsT=w_sb_blocks[:, kc, ts(mc, 128)],
                                    rhs=h1_sb[kc // 2][:, kc % 2, :],
                                    start=(kc == 0),
                                    stop=(kc == HC - 1 and not use_bias),
                                )
                        if use_bias:
                            brow = ba_sb if ps is a_ps else bb2_sb
                            for mc in range(DC):
                                nc.tensor.matmul(
                                    ps[:, mc, :],
                                    lhsT=brow[0:1, ts(mc, 128)],
                                    rhs=ones_sb[0:1, :],
                                    start=False,
                                    stop=True,
                                )
                    a_sb = actp.tile([128, DC, T], BF16)
                    nc.scalar.activation(a_sb, a_ps, mybir.ActivationFunctionType.Tanh)
                    g_sb = actp.tile([128, DC, T], BF16)
                    nc.scalar.activation(g_sb, g_ps, mybir.ActivationFunctionType.Tanh)

                    # ---- s' = a*(1+g)  (= 2*a*sigmoid-gate) ----
                    gp1 = actp.tile([128, DC, T], BF16)
                    nc.vector.tensor_scalar_add(gp1, g_sb, 1.0)
                    s_sb = actp.tile([128, DC, T], BF16)
                    nc.vector.tensor_mul(s_sb, a_sb, gp1)

                    prev = (bag, t, a_ps, s_sb, h1_sb)

            # flush the final tile's score path (emits the last bag's DMAs)
            score_path(prev)
    nc.finalize()
    return nc


def kernel(h, W1, b1, Wa, ba, Wb, bb, Wc, bc, Wcls, bcls):
    global LAST_EXEC_NS
    h = np.asarray(h, dtype=np.float32)
    W1 = np.asarray(W1, dtype=np.float32)
    b1 = np.asarray(b1, dtype=np.float32)
    Wa = np.asarray(Wa, dtype=np.float32)
    ba = np.asarray(ba, dtype=np.float32)
    Wb = np.asarray(Wb, dtype=np.float32)
    bb = np.asarray(bb, dtype=np.float32)
    Wc = np.asarray(Wc, dtype=np.float32)
    bc = np.asarray(bc, dtype=np.float32)
    Wcls = np.asarray(Wcls, dtype=np.float32)
    bcls = np.asarray(bcls, dtype=np.float32)

    bf = ml_dtypes.bfloat16
    use_bias = bool(np.any(b1) or np.any(ba) or np.any(bb))

    key = use_bias
    if key not in _CACHE:
        _CACHE[key] = _build(use_bias)
    nc = _CACHE[key]

    # host-side input prep: transpose h to [bag, F, N], fold the
    # sigmoid->tanh rewrite into Wb/bb, replicate Wc/2 across 128 cols
    hT = h.transpose(0, 2, 1).reshape(B, FC, 128, NT, T)
    hT = np.ascontiguousarray(hT.transpose(0, 3, 2, 1, 4)).astype(bf)
    w1_b = W1.astype(bf)
    wa_b = Wa.astype(bf)
    wb2_b = (Wb * 0.5).astype(bf)
    wc2r_b = np.ascontiguousarray(np.repeat(Wc * 0.5, 128, axis=1)).astype(bf)

    in_maps = []
    for c in range(NCORES):
        m = {
            "h": hT[c * BAGS_PER_CORE : (c + 1) * BAGS_PER_CORE],
            "w1": w1_b,
            "wa": wa_b,
            "wb2": wb2_b,
            "wc2r": wc2r_b,
        }
        if use_bias:
            m["b1r"] = b1.reshape(1, H).astype(bf)
            m["bar"] = ba.reshape(1, D).astype(bf)
            m["bb2r"] = (bb * 0.5).reshape(1, D).astype(bf)
        in_maps.append(m)

    res = run_bass_kernel_spmd(
        nc, in_maps, core_ids=list(range(NCORES)), trace=PROFILE
    )
    LAST_EXEC_NS = res.exec_time_ns
    outs = res.results

    A_dev = np.concatenate([o["a_out"] for o in outs], axis=0).astype(np.float32)
    S_dev = np.concatenate([o["s_out"] for o in outs], axis=0)  # [16, 128, HC]
    S_dev = np.ascontiguousarray(S_dev.transpose(0, 2, 1)).reshape(B, H)

    # host epilogue (the "[1,512] all-reduce + classifier" in the hint)
    # softmax weights on device were bf16(exp(A)); reproduce for Z
    w_host = np.exp(A_dev).astype(bf).astype(np.float32)
    Z = w_host.sum(axis=1, keepdims=True)  # [16, 1]
    M = (S_dev / Z).astype(np.float32)  # [16, 512]

    A_raw = (A_dev + bc[0]).reshape(B, 1, N).astype(np.float32)
    M_out = M.reshape(B, 1, H)
    context = M.mean(axis=0, keepdims=True)  # [1, 512]
    logits = (context @ Wcls + bcls).astype(np.float32)  # [1, 2]
    ex = np.exp(logits - logits.max(axis=1, keepdims=True))
    Y_prob = (ex / ex.sum(axis=1, keepdims=True)).astype(np.float32)
    Y_hat = np.argmax(logits, axis=1, keepdims=True).astype(np.int32)
    return (logits, Y_prob, Y_hat, A_raw, M_out)


# revision 20
# speedup vs baseline: 1.3602x; 1.0030x over previous
"""Trainium2 Bass kernel for the CARP3D attention-MIL pooling model.

Model (per bag b of B=16, N=8192 instances, F=512 features):
    h1 = relu(h @ W1 + b1)            [B,N,H]   H=512
    a  = tanh(h1 @ Wa + ba)           [B,N,D]   D=256
    g  = sigmoid(h1 @ Wb + bb)        [B,N,D]
    A  = (a*g) @ Wc + bc              [B,N,1]
    A_sm = softmax(A over N)
    M  = A_sm @ h1                    [B,1,H]
    context = mean_b M; logits = context @ Wcls + bcls

Distribution: data-parallel over bags, 2 bags per core on 8 cores.

Layout strategy: the host pre-transposes h to [bag, F, N] bf16 so the
contraction dim (features) is on SBUF partitions for every matmul and no
on-device transposes are needed.  All compute stays in "feature-major"
space:
    h1T[H,n] = relu(W1.T @ hT)       (lhsT = W1 blocks, rhs = hT)
    aT[D,n]  = tanh(Wa.T @ h1T)
    gT       = tanh(0.5*Wb.T @ h1T)  (sigmoid(x) = 0.5*(1+tanh(x/2)))
    s'       = aT + aT*gT            (= 2*a*g)
    A_rep    = (0.5*Wc replicated 128x).T @ s'   -> [128, n] with every
               partition holding the same score row (free broadcast)
    w        = exp(A_rep)            (no max-subtraction: |A| << 1 here)
    S[H]    += sum_n h1T[H,n]*w[n]   (DVE mul + free-dim reduce)
The per-tile score chain (tanh -> s' -> A -> exp) is software-pipelined
one tile behind the matmul stream so TensorE runs gap-free.
Outputs per core: raw score rows A (bf16) and unnormalized pooled S (f32).
The softmax normalization (Z), bc shift (softmax-invariant), mean over
bags and the [1,512]@[512,2] classifier run on host - microseconds of
work vs the 256MB streaming on device.
"""

import os
import sys

import numpy as np
import ml_dtypes

for _p in ("/opt/trn_rl_repo",):
    if _p not in sys.path:
        sys.path.insert(0, _p)

import concourse.bass as bass
import concourse.mybir as mybir
import concourse.tile as tile
from concourse import bacc
from concourse.bass import ts
from concourse.bass_utils import run_bass_kernel_spmd

BF16 = mybir.dt.bfloat16
F32 = mybir.dt.float32

B, N, F, H, D = 16, 8192, 512, 512, 256
NCORES = 8
BAGS_PER_CORE = B // NCORES          # 2
T = 512                              # instance tile size
NT = N // T                          # 16 tiles per bag
FC = F // 128                        # 4 F-chunks
HC = H // 128                        # 4 H-chunks
DC = D // 128                        # 2 D-chunks

# set by test.py to collect a hardware profile
PROFILE = False
LAST_EXEC_NS = None
_CACHE = {}


def _build(use_bias: bool):
    nc = bacc.Bacc("TRN2", target_bir_lowering=False)

    # h layout: [bag, n_tile, partition, f_chunk, n_in_tile] so each
    # partition's per-tile data is one contiguous 4KB run (fast descriptors)
    h_d = nc.dram_tensor("h", [BAGS_PER_CORE, NT, 128, FC, T], BF16, kind="ExternalInput")
    w1_d = nc.dram_tensor("w1", [F, H], BF16, kind="ExternalInput")
    wa_d = nc.dram_tensor("wa", [H, D], BF16, kind="ExternalInput")
    wb2_d = nc.dram_tensor("wb2", [H, D], BF16, kind="ExternalInput")
    wc2r_d = nc.dram_tensor("wc2r", [D, 128], BF16, kind="ExternalInput")
    if use_bias:
        b1_d = nc.dram_tensor("b1r", [1, H], BF16, kind="ExternalInput")
        ba_d = nc.dram_tensor("bar", [1, D], BF16, kind="ExternalInput")
        bb2_d = nc.dram_tensor("bb2r", [1, D], BF16, kind="ExternalInput")
    a_out = nc.dram_tensor("a_out", [BAGS_PER_CORE, N], BF16, kind="ExternalOutput")
    # partition-major so the DMA writes 16B runs, not 4B scatter
    s_out = nc.dram_tensor("s_out", [BAGS_PER_CORE, 128, HC], F32, kind="ExternalOutput")

    with tile.TileContext(nc) as tc:
        with (
            tc.tile_pool(name="const", bufs=1) as const,
            tc.tile_pool(name="hin", bufs=4) as hin,
            tc.tile_pool(name="h1pool", bufs=2) as h1pool,
            tc.tile_pool(name="acts", bufs=2) as actp,
            tc.tile_pool(name="wexp", bufs=2) as wexp,
            tc.tile_pool(name="scratch", bufs=1) as scratch,
            tc.tile_pool(name="spart", bufs=2) as spartp,
            tc.tile_pool(name="sacc", bufs=2) as saccp,
            tc.tile_pool(name="arow", bufs=2) as arowp,
            tc.tile_pool(name="h1psum", bufs=1, space="PSUM") as h1psum,
            tc.tile_pool(name="agpsum", bufs=1, space="PSUM") as agpsum,
        ):
            # --- weights, loaded once (w1 + first h tile first: they gate
            # the first matmul; the rest follows) ---
            w1_sb = const.tile([128, FC, H], BF16)
            ht0 = hin.tile([128, FC, T], BF16, tag="ht", name="ht")
            w1_r = w1_d.rearrange("(kc p) h -> p kc h", p=128)
            for kc in range(FC):
                nc.sync.dma_start(out=w1_sb[:, kc, :], in_=w1_r[:, kc, :])
                nc.sync.dma_start(out=ht0[:, kc, :], in_=h_d[0, 0, :, kc, :])
            wa_sb = const.tile([128, HC, D], BF16)
            nc.sync.dma_start(out=wa_sb, in_=wa_d.rearrange("(kc p) d -> p kc d", p=128))
            wb2_sb = const.tile([128, HC, D], BF16)
            nc.sync.dma_start(out=wb2_sb, in_=wb2_d.rearrange("(kc p) d -> p kc d", p=128))
            wc2r_sb = const.tile([128, DC, 128], BF16)
            nc.sync.dma_start(out=wc2r_sb, in_=wc2r_d.rearrange("(kc p) m -> p kc m", p=128))
            if use_bias:
                b1_sb = const.tile([1, H], BF16)
                nc.sync.dma_start(out=b1_sb, in_=b1_d)
                ba_sb = const.tile([1, D], BF16)
                nc.sync.dma_start(out=ba_sb, in_=ba_d)
                bb2_sb = const.tile([1, D], BF16)
                nc.sync.dma_start(out=bb2_sb, in_=bb2_d)
                ones_sb = const.tile([1, T], BF16)
                nc.vector.memset(ones_sb, 1.0)

            s_accs = [saccp.tile([128, HC], F32, name=f"s_acc{b}") for b in range(BAGS_PER_CORE)]
            a_rows = [arowp.tile([1, N], BF16, name=f"a_row{b}") for b in range(BAGS_PER_CORE)]

            # score path of tile t-1, emitted during tile t so exp / a_row /
            # pooling hide behind tile t's h1 matmuls; carried across the
            # bag boundary so bags never serialize.  Emits the bag's output
            # DMAs when its last tile retires.
            if True:
                def score_path(prev):
                    pb, pt, p_ag, p_s, p_h1 = prev  # p_ag = that tile's a_ps
                    s_acc = s_accs[pb]
                    a_row = a_rows[pb]
                    for kc in range(DC):
                        nc.tensor.matmul(
                            p_ag[:, 0, :],
                            lhsT=wc2r_sb[:, kc, :],
                            rhs=p_s[:, kc, :],
                            start=(kc == 0),
                            stop=(kc == DC - 1),
                        )
                    arep = wexp.tile([128, T], BF16, tag="arep", name="arep")
                    nc.vector.tensor_copy(out=arep, in_=p_ag[:, 0, :])
                    w_sb = wexp.tile([128, T], BF16)
                    nc.scalar.activation(w_sb, arep, mybir.ActivationFunctionType.Exp)
                    nc.vector.tensor_copy(out=a_row[0:1, ts(pt, T)], in_=arep[0:1, :])
                    wh = scratch.tile([128, HC, T], BF16)
                    w_bc2 = bass.AP(
                        tensor=w_sb.tensor,
                        offset=w_sb.offset,
                        ap=[w_sb.ap[0], [0, 2], w_sb.ap[1]],
                    )
                    nc.vector.tensor_mul(wh[:, 0:2, :], p_h1[0], w_bc2)
                    nc.vector.tensor_mul(wh[:, 2:4, :], p_h1[1], w_bc2)
                    s_part = spartp.tile([128, HC], F32)
                    nc.vector.tensor_reduce(
                        out=s_part,
                        in_=wh,
                        axis=mybir.AxisListType.X,
                        op=mybir.AluOpType.add,
                    )
                    if pt == 0:
                        nc.vector.tensor_copy(out=s_acc, in_=s_part)
                    else:
                        nc.vector.tensor_add(s_acc, s_acc, s_part)
                    if pt == NT - 1:
                        nc.sync.dma_start(out=a_out[pb : pb + 1, :], in_=a_row)
                        nc.sync.dma_start(out=s_out[pb], in_=s_acc)

            prev = None
            for bag in range(BAGS_PER_CORE):
                for t in range(NT):
                    # ---- load hT tile [F=4x128, T] ----
                    if bag == 0 and t == 0:
                        ht = ht0
                    else:
                        ht = hin.tile([128, FC, T], BF16, tag="ht", name="ht")
                        nc.sync.dma_start(out=ht, in_=h_d[bag, t])

                    # ---- h1T = relu(W1.T @ hT) ----
                    # Tile tracks deps at whole-tile granularity, so the two
                    # halves live in separate tiles: relu/ag-matmuls on half
                    # A overlap the half-B matmuls.
                    h1_ps = [
                        h1psum.tile([128, 2, T], F32, tag="h1ps_a", name="h1ps_a"),
                        h1psum.tile([128, 2, T], F32, tag="h1ps_b", name="h1ps_b"),
                    ]
                    # tile 0: kc-outer so the first matmul only needs the
                    # first w1/ht chunk DMAs (head latency); later tiles:
                    # mc-outer so relu can start at the half-tile boundary
                    if bag == 0 and t == 0:
                        mmorder = [(mc, kc) for kc in range(FC) for mc in range(HC)]
                    else:
                        mmorder = [(mc, kc) for mc in range(HC) for kc in range(FC)]
                    for mc, kc in mmorder:
                        nc.tensor.matmul(
                            h1_ps[mc // 2][:, mc % 2, :],
                            lhsT=w1_sb[:, kc, ts(mc, 128)],
                            rhs=ht[:, kc, :],
                            start=(kc == 0),
                            stop=(kc == FC - 1 and not use_bias),
                        )
                        if use_bias:
                            nc.tensor.matmul(
                                h1_ps[mc // 2][:, mc % 2, :],
                                lhsT=b1_sb[0:1, ts(mc, 128)],
                                rhs=ones_sb[0:1, :],
                                start=False,
                                stop=True,
                            )
                    # previous tile's score path runs here, hidden behind
                    # the h1 matmuls above
                    if prev is not None:
                        score_path(prev)

                    h1_sb = [
                        h1pool.tile([128, 2, T], BF16, tag="h1sb_a", name="h1sb_a"),
                        h1pool.tile([128, 2, T], BF16, tag="h1sb_b", name="h1sb_b"),
                    ]
                    nc.scalar.activation(h1_sb[0], h1_ps[0], mybir.ActivationFunctionType.Relu)
                    nc.scalar.activation(h1_sb[1], h1_ps[1], mybir.ActivationFunctionType.Relu)

                    # ---- aT / gT pre-activations (k-outer: consume h1
                    # chunks as they become ready) ----
                    # a and g in separate psum tiles, all a-matmuls first:
                    # tanh(a) runs while the g-matmuls stream, shortening the
                    # per-tile critical chain
                    a_ps = agpsum.tile([128, DC, T], F32, tag="a_ps", name="a_ps")
                    g_ps = agpsum.tile([128, DC, T], F32, tag="g_ps", name="g_ps")
                    for ps, w_sb_blocks, brow_name in ((a_ps, wa_sb, "ba"), (g_ps, wb2_sb, "bb2")):
                        for kc in range(HC):
                            for mc in range(DC):
                                nc.tensor.matmul(
                                    ps[:, mc, :],
                                    lhsT=w_sb_blocks[:, kc, ts(mc, 128)],
                                    rhs=h1_sb[kc // 2][:, kc % 2, :],
                                    start=(kc == 0),
                                    stop=(kc == HC - 1 and not use_bias),
                                )
                        if use_bias:
                            brow = ba_sb if ps is a_ps else bb2_sb
                            for mc in range(DC):
                                nc.tensor.matmul(
                                    ps[:, mc, :],
                                    lhsT=brow[0:1, ts(mc, 128)],
                                    rhs=ones_sb[0:1, :],
                                    start=False,
                                    stop=True,
                                )
                    a_sb = actp.tile([128, DC, T], BF16)
                    nc.scalar.activation(a_sb, a_ps, mybir.ActivationFunctionType.Tanh)
                    g_sb = actp.tile([128, DC, T], BF16)
                    nc.scalar.activation(g_sb, g_ps, mybir.ActivationFunctionType.Tanh)

                    # ---- s' = a*(1+g)  (= 2*a*sigmoid-gate) ----
                    gp1 = actp.tile([128, DC, T], BF16)
                    nc.vector.tensor_scalar_add(gp1, g_sb, 1.0)
                    s_sb = actp.tile([128, DC, T], BF16)
                    nc.vector.tensor_mul(s_sb, a_sb, gp1)

                    prev = (bag, t, a_ps, s_sb, h1_sb)

            # flush the final tile's score path (emits the last bag's DMAs)
            score_path(prev)
    nc.finalize()
    return nc


def kernel(h, W1, b1, Wa, ba, Wb, bb, Wc, bc, Wcls, bcls):
    global LAST_EXEC_NS
    h = np.asarray(h, dtype=np.float32)
    W1 = np.asarray(W1, dtype=np.float32)
    b1 = np.asarray(b1, dtype=np.float32)
    Wa = np.asarray(Wa, dtype=np.float32)
    ba = np.asarray(ba, dtype=np.float32)
    Wb = np.asarray(Wb, dtype=np.float32)
    bb = np.asarray(bb, dtype=np.float32)
    Wc = np.asarray(Wc, dtype=np.float32)
    bc = np.asarray(bc, dtype=np.float32)
    Wcls = np.asarray(Wcls, dtype=np.float32)
    bcls = np.asarray(bcls, dtype=np.float32)

    bf = ml_dtypes.bfloat16
    use_bias = bool(np.any(b1) or np.any(ba) or np.any(bb))

    key = use_bias
    if key not in _CACHE:
        _CACHE[key] = _build(use_bias)
    nc = _CACHE[key]

    # host-side input prep: transpose h to [bag, F, N], fold the
    # sigmoid->tanh rewrite into Wb/bb, replicate Wc/2 across 128 cols
    hT = h.transpose(0, 2, 1).reshape(B, FC, 128, NT, T)
    hT = np.ascontiguousarray(hT.transpose(0, 3, 2, 1, 4)).astype(bf)
    w1_b = W1.astype(bf)
    wa_b = Wa.astype(bf)
    wb2_b = (Wb * 0.5).astype(bf)
    wc2r_b = np.ascontiguousarray(np.repeat(Wc * 0.5, 128, axis=1)).astype(bf)

    in_maps = []
    for c in range(NCORES):
        m = {
            "h": hT[c * BAGS_PER_CORE : (c + 1) * BAGS_PER_CORE],
            "w1": w1_b,
            "wa": wa_b,
            "wb2": wb2_b,
            "wc2r": wc2r_b,
        }
        if use_bias:
            m["b1r"] = b1.reshape(1, H).astype(bf)
            m["bar"] = ba.reshape(1, D).astype(bf)
            m["bb2r"] = (bb * 0.5).reshape(1, D).astype(bf)
        in_maps.append(m)

    res = run_bass_kernel_spmd(
        nc, in_maps, core_ids=list(range(NCORES)), trace=PROFILE
    )
    LAST_EXEC_NS = res.exec_time_ns
    outs = res.results

    A_dev = np.concatenate([o["a_out"] for o in outs], axis=0).astype(np.float32)
    S_dev = np.concatenate([o["s_out"] for o in outs], axis=0)  # [16, 128, HC]
    S_dev = np.ascontiguousarray(S_dev.transpose(0, 2, 1)).reshape(B, H)

    # host epilogue (the "[1,512] all-reduce + classifier" in the hint)
    # softmax weights on device were bf16(exp(A)); reproduce for Z
    w_host = np.exp(A_dev).astype(bf).astype(np.float32)
    Z = w_host.sum(axis=1, keepdims=True)  # [16, 1]
    M = (S_dev / Z).astype(np.float32)  # [16, 512]

    A_raw = (A_dev + bc[0]).reshape(B, 1, N).astype(np.float32)
    M_out = M.reshape(B, 1, H)
    context = M.mean(axis=0, keepdims=True)  # [1, 512]
    logits = (context @ Wcls + bcls).astype(np.float32)  # [1, 2]
    ex = np.exp(logits - logits.max(axis=1, keepdims=True))
    Y_prob = (ex / ex.sum(axis=1, keepdims=True)).astype(np.float32)
    Y_hat = np.argmax(logits, axis=1, keepdims=True).astype(np.int32)
    return (logits, Y_prob, Y_hat, A_raw, M_out)


# revision 21
# speedup vs baseline: 1.3628x; 1.0019x over previous
"""Trainium2 Bass kernel for the CARP3D attention-MIL pooling model.

Model (per bag b of B=16, N=8192 instances, F=512 features):
    h1 = relu(h @ W1 + b1)            [B,N,H]   H=512
    a  = tanh(h1 @ Wa + ba)           [B,N,D]   D=256
    g  = sigmoid(h1 @ Wb + bb)        [B,N,D]
    A  = (a*g) @ Wc + bc              [B,N,1]
    A_sm = softmax(A over N)
    M  = A_sm @ h1                    [B,1,H]
    context = mean_b M; logits = context @ Wcls + bcls

Distribution: data-parallel over bags, 2 bags per core on 8 cores.

Layout strategy: the host pre-transposes h to [bag, F, N] bf16 so the
contraction dim (features) is on SBUF partitions for every matmul and no
on-device transposes are needed.  All compute stays in "feature-major"
space:
    h1T[H,n] = relu(W1.T @ hT)       (lhsT = W1 blocks, rhs = hT)
    aT[D,n]  = tanh(Wa.T @ h1T)
    gT       = tanh(0.5*Wb.T @ h1T)  (sigmoid(x) = 0.5*(1+tanh(x/2)))
    s'       = aT + aT*gT            (= 2*a*g)
    A_rep    = (0.5*Wc replicated 128x).T @ s'   -> [128, n] with every
               partition holding the same score row (free broadcast)
    w        = exp(A_rep)            (no max-subtraction: |A| << 1 here)
    S[H]    += sum_n h1T[H,n]*w[n]   (DVE mul + free-dim reduce)
The per-tile score chain (tanh -> s' -> A -> exp) is software-pipelined
one tile behind the matmul stream so TensorE runs gap-free.
Outputs per core: raw score rows A (bf16) and unnormalized pooled S (f32).
The softmax normalization (Z), bc shift (softmax-invariant), mean over
bags and the [1,512]@[512,2] classifier run on host - microseconds of
work vs the 256MB streaming on device.
"""

import os
import sys

import numpy as np
import ml_dtypes

for _p in ("/opt/trn_rl_repo",):
    if _p not in sys.path:
        sys.path.insert(0, _p)

import concourse.bass as bass
import concourse.mybir as mybir
import concourse.tile as tile
from concourse import bacc
from concourse.bass import ts
from concourse.bass_utils import run_bass_kernel_spmd

BF16 = mybir.dt.bfloat16
F32 = mybir.dt.float32

B, N, F, H, D = 16, 8192, 512, 512, 256
NCORES = 8
BAGS_PER_CORE = B // NCORES          # 2
T = 512                              # instance tile size
NT = N // T                          # 16 tiles per bag
FC = F // 128                        # 4 F-chunks
HC = H // 128                        # 4 H-chunks
DC = D // 128                        # 2 D-chunks

# set by test.py to collect a hardware profile
PROFILE = False
LAST_EXEC_NS = None
_CACHE = {}


def _build(use_bias: bool):
    nc = bacc.Bacc("TRN2", target_bir_lowering=False)

    # h layout: [bag, n_tile, partition, f_chunk, n_in_tile] so each
    # partition's per-tile data is one contiguous 4KB run (fast descriptors)
    h_d = nc.dram_tensor("h", [BAGS_PER_CORE, NT, 128, FC, T], BF16, kind="ExternalInput")
    w1_d = nc.dram_tensor("w1", [F, H], BF16, kind="ExternalInput")
    wa_d = nc.dram_tensor("wa", [H, D], BF16, kind="ExternalInput")
    wb2_d = nc.dram_tensor("wb2", [H, D], BF16, kind="ExternalInput")
    wc2r_d = nc.dram_tensor("wc2r", [D, 128], BF16, kind="ExternalInput")
    if use_bias:
        b1_d = nc.dram_tensor("b1r", [1, H], BF16, kind="ExternalInput")
        ba_d = nc.dram_tensor("bar", [1, D], BF16, kind="ExternalInput")
        bb2_d = nc.dram_tensor("bb2r", [1, D], BF16, kind="ExternalInput")
    a_out = nc.dram_tensor("a_out", [BAGS_PER_CORE, N], BF16, kind="ExternalOutput")
    # partition-major so the DMA writes 16B runs, not 4B scatter
    s_out = nc.dram_tensor("s_out", [BAGS_PER_CORE, 128, HC], F32, kind="ExternalOutput")

    with tile.TileContext(nc) as tc:
        with (
            tc.tile_pool(name="const", bufs=1) as const,
            tc.tile_pool(name="hin", bufs=6) as hin,
            tc.tile_pool(name="h1pool", bufs=2) as h1pool,
            tc.tile_pool(name="acts", bufs=3) as actp,
            tc.tile_pool(name="wexp", bufs=3) as wexp,
            tc.tile_pool(name="scratch", bufs=1) as scratch,
            tc.tile_pool(name="spart", bufs=2) as spartp,
            tc.tile_pool(name="sacc", bufs=2) as saccp,
            tc.tile_pool(name="arow", bufs=2) as arowp,
            tc.tile_pool(name="h1psum", bufs=1, space="PSUM") as h1psum,
            tc.tile_pool(name="agpsum", bufs=1, space="PSUM") as agpsum,
        ):
            # --- weights, loaded once (w1 + first h tile first: they gate
            # the first matmul; the rest follows) ---
            w1_sb = const.tile([128, FC, H], BF16)
            ht0 = hin.tile([128, FC, T], BF16, tag="ht", name="ht")
            w1_r = w1_d.rearrange("(kc p) h -> p kc h", p=128)
            for kc in range(FC):
                nc.sync.dma_start(out=w1_sb[:, kc, :], in_=w1_r[:, kc, :])
                nc.scalar.dma_start(out=ht0[:, kc, :], in_=h_d[0, 0, :, kc, :])
            wa_sb = const.tile([128, HC, D], BF16)
            nc.sync.dma_start(out=wa_sb, in_=wa_d.rearrange("(kc p) d -> p kc d", p=128))
            wb2_sb = const.tile([128, HC, D], BF16)
            nc.sync.dma_start(out=wb2_sb, in_=wb2_d.rearrange("(kc p) d -> p kc d", p=128))
            wc2r_sb = const.tile([128, DC, 128], BF16)
            nc.sync.dma_start(out=wc2r_sb, in_=wc2r_d.rearrange("(kc p) m -> p kc m", p=128))
            if use_bias:
                b1_sb = const.tile([1, H], BF16)
                nc.sync.dma_start(out=b1_sb, in_=b1_d)
                ba_sb = const.tile([1, D], BF16)
                nc.sync.dma_start(out=ba_sb, in_=ba_d)
                bb2_sb = const.tile([1, D], BF16)
                nc.sync.dma_start(out=bb2_sb, in_=bb2_d)
                ones_sb = const.tile([1, T], BF16)
                nc.vector.memset(ones_sb, 1.0)

            s_accs = [saccp.tile([128, HC], F32, name=f"s_acc{b}") for b in range(BAGS_PER_CORE)]
            a_rows = [arowp.tile([1, N], BF16, name=f"a_row{b}") for b in range(BAGS_PER_CORE)]

            # score path of tile t-1, emitted during tile t so exp / a_row /
            # pooling hide behind tile t's h1 matmuls; carried across the
            # bag boundary so bags never serialize.  Emits the bag's output
            # DMAs when its last tile retires.
            if True:
                def score_path(prev):
                    pb, pt, p_ag, p_s, p_h1 = prev  # p_ag = that tile's a_ps
                    s_acc = s_accs[pb]
                    a_row = a_rows[pb]
                    for kc in range(DC):
                        nc.tensor.matmul(
                            p_ag[:, 0, :],
                            lhsT=wc2r_sb[:, kc, :],
                            rhs=p_s[:, kc, :],
                            start=(kc == 0),
                            stop=(kc == DC - 1),
                        )
                    arep = wexp.tile([128, T], BF16, tag="arep", name="arep")
                    nc.vector.tensor_copy(out=arep, in_=p_ag[:, 0, :])
                    w_sb = wexp.tile([128, T], BF16)
                    nc.scalar.activation(w_sb, arep, mybir.ActivationFunctionType.Exp)
                    nc.vector.tensor_copy(out=a_row[0:1, ts(pt, T)], in_=arep[0:1, :])
                    wh = scratch.tile([128, HC, T], BF16)
                    w_bc2 = bass.AP(
                        tensor=w_sb.tensor,
                        offset=w_sb.offset,
                        ap=[w_sb.ap[0], [0, 2], w_sb.ap[1]],
                    )
                    nc.vector.tensor_mul(wh[:, 0:2, :], p_h1[0], w_bc2)
                    nc.vector.tensor_mul(wh[:, 2:4, :], p_h1[1], w_bc2)
                    s_part = spartp.tile([128, HC], F32)
                    nc.vector.tensor_reduce(
                        out=s_part,
                        in_=wh,
                        axis=mybir.AxisListType.X,
                        op=mybir.AluOpType.add,
                    )
                    if pt == 0:
                        nc.vector.tensor_copy(out=s_acc, in_=s_part)
                    else:
                        nc.vector.tensor_add(s_acc, s_acc, s_part)
                    if pt == NT - 1:
                        nc.sync.dma_start(out=a_out[pb : pb + 1, :], in_=a_row)
                        nc.sync.dma_start(out=s_out[pb], in_=s_acc)

            prev = None
            for bag in range(BAGS_PER_CORE):
                for t in range(NT):
                    # ---- load hT tile [F=4x128, T] ----
                    if bag == 0 and t == 0:
                        ht = ht0
                    else:
                        ht = hin.tile([128, FC, T], BF16, tag="ht", name="ht")
                        nc.sync.dma_start(out=ht, in_=h_d[bag, t])

                    # ---- h1T = relu(W1.T @ hT) ----
                    # Tile tracks deps at whole-tile granularity, so the two
                    # halves live in separate tiles: relu/ag-matmuls on half
                    # A overlap the half-B matmuls.
                    h1_ps = [
                        h1psum.tile([128, 2, T], F32, tag="h1ps_a", name="h1ps_a"),
                        h1psum.tile([128, 2, T], F32, tag="h1ps_b", name="h1ps_b"),
                    ]
                    # tile 0: kc-outer so the first matmul only needs the
                    # first w1/ht chunk DMAs (head latency); later tiles:
                    # mc-outer so relu can start at the half-tile boundary
                    if bag == 0 and t == 0:
                        mmorder = [(mc, kc) for kc in range(FC) for mc in range(HC)]
                    else:
                        mmorder = [(mc, kc) for mc in range(HC) for kc in range(FC)]
                    for mc, kc in mmorder:
                        nc.tensor.matmul(
                            h1_ps[mc // 2][:, mc % 2, :],
                            lhsT=w1_sb[:, kc, ts(mc, 128)],
                            rhs=ht[:, kc, :],
                            start=(kc == 0),
                            stop=(kc == FC - 1 and not use_bias),
                        )
                        if use_bias:
                            nc.tensor.matmul(
                                h1_ps[mc // 2][:, mc % 2, :],
                                lhsT=b1_sb[0:1, ts(mc, 128)],
                                rhs=ones_sb[0:1, :],
                                start=False,
                                stop=True,
                            )
                    # previous tile's score path runs here, hidden behind
                    # the h1 matmuls above
                    if prev is not None:
                        score_path(prev)

                    h1_sb = [
                        h1pool.tile([128, 2, T], BF16, tag="h1sb_a", name="h1sb_a"),
                        h1pool.tile([128, 2, T], BF16, tag="h1sb_b", name="h1sb_b"),
                    ]
                    nc.scalar.activation(h1_sb[0], h1_ps[0], mybir.ActivationFunctionType.Relu)
                    nc.scalar.activation(h1_sb[1], h1_ps[1], mybir.ActivationFunctionType.Relu)

                    # ---- aT / gT pre-activations (k-outer: consume h1
                    # chunks as they become ready) ----
                    # a and g in separate psum tiles, all a-matmuls first:
                    # tanh(a) runs while the g-matmuls stream, shortening the
                    # per-tile critical chain
                    a_ps = agpsum.tile([128, DC, T], F32, tag="a_ps", name="a_ps")
                    g_ps = agpsum.tile([128, DC, T], F32, tag="g_ps", name="g_ps")
                    for ps, w_sb_blocks, brow_name in ((a_ps, wa_sb, "ba"), (g_ps, wb2_sb, "bb2")):
                        for kc in range(HC):
                            for mc in range(DC):
                                nc.tensor.matmul(
                                    ps[:, mc, :],
                                    lhsT=w_sb_blocks[:, kc, ts(mc, 128)],
                                    rhs=h1_sb[kc // 2][:, kc % 2, :],
                                    start=(kc == 0),
                                    stop=(kc == HC - 1 and not use_bias),
                                )
                        if use_bias:
                            brow = ba_sb if ps is a_ps else bb2_sb
                            for mc in range(DC):
                                nc.tensor.matmul(
                                    ps[:, mc, :],
                                    lhsT=brow[0:1, ts(mc, 128)],
                                    rhs=ones_sb[0:1, :],
                                    start=False,
                                    stop=True,
                                )
                    a_sb = actp.tile([128, DC, T], BF16)
                    nc.scalar.activation(a_sb, a_ps, mybir.ActivationFunctionType.Tanh)
                    g_sb = actp.tile([128, DC, T], BF16)
                    nc.scalar.activation(g_sb, g_ps, mybir.ActivationFunctionType.Tanh)

                    # ---- s' = a*(1+g)  (= 2*a*sigmoid-gate) ----
                    gp1 = actp.tile([128, DC, T], BF16)
                    nc.vector.tensor_scalar_add(gp1, g_sb, 1.0)
                    s_sb = actp.tile([128, DC, T], BF16)
                    nc.vector.tensor_mul(s_sb, a_sb, gp1)

                    prev = (bag, t, a_ps, s_sb, h1_sb)

            # flush the final tile's score path (emits the last bag's DMAs)
            score_path(prev)
    nc.finalize()
    return nc


def kernel(h, W1, b1, Wa, ba, Wb, bb, Wc, bc, Wcls, bcls):
    global LAST_EXEC_NS
    h = np.asarray(h, dtype=np.float32)
    W1 = np.asarray(W1, dtype=np.float32)
    b1 = np.asarray(b1, dtype=np.float32)
    Wa = np.asarray(Wa, dtype=np.float32)
    ba = np.asarray(ba, dtype=np.float32)
    Wb = np.asarray(Wb, dtype=np.float32)
    bb = np.asarray(bb, dtype=np.float32)
    Wc = np.asarray(Wc, dtype=np.float32)
    bc = np.asarray(bc, dtype=np.float32)
    Wcls = np.asarray(Wcls, dtype=np.float32)
    bcls = np.asarray(bcls, dtype=np.float32)

    bf = ml_dtypes.bfloat16
    use_bias = bool(np.any(b1) or np.any(ba) or np.any(bb))

    key = use_bias
    if key not in _CACHE:
        _CACHE[key] = _build(use_bias)
    nc = _CACHE[key]

    # host-side input prep: transpose h to [bag, F, N], fold the
    # sigmoid->tanh rewrite into Wb/bb, replicate Wc/2 across 128 cols
    hT = h.transpose(0, 2, 1).reshape(B, FC, 128, NT, T)
    hT = np.ascontiguousarray(hT.transpose(0, 3, 2, 1, 4)).astype(bf)
    w1_b = W1.astype(bf)
    wa_b = Wa.astype(bf)
    wb2_b = (Wb * 0.5).astype(bf)
    wc2r_b = np.ascontiguousarray(np.repeat(Wc * 0.5, 128, axis=1)).astype(bf)

    in_maps = []
    for c in range(NCORES):
        m = {
            "h": hT[c * BAGS_PER_CORE : (c + 1) * BAGS_PER_CORE],
            "w1": w1_b,
            "wa": wa_b,
            "wb2": wb2_b,
            "wc2r": wc2r_b,
        }
        if use_bias:
            m["b1r"] = b1.reshape(1, H).astype(bf)
            m["bar"] = ba.reshape(1, D).astype(bf)
            m["bb2r"] = (bb * 0.5).reshape(1, D).astype(bf)
        in_maps.append(m)

    res = run_bass_kernel_spmd(
        nc, in_maps, core_ids=list(range(NCORES)), trace=PROFILE
    )
    LAST_EXEC_NS = res.exec_time_ns
    outs = res.results

    A_dev = np.concatenate([o["a_out"] for o in outs], axis=0).astype(np.float32)
    S_dev = np.concatenate([o["s_out"] for o in outs], axis=0)  # [16, 128, HC]
    S_dev = np.ascontiguousarray(S_dev.transpose(0, 2, 1)).reshape(B, H)

    # host epilogue (the "[1,512] all-reduce + classifier" in the hint)
    # softmax weights on device were bf16(exp(A)); reproduce for Z
    w_host = np.exp(A_dev).astype(bf).astype(np.float32)
    Z = w_host.sum(axis=1, keepdims=True)  # [16, 1]
    M = (S_dev / Z).astype(np.float32)  # [16, 512]

    A_raw = (A_dev + bc[0]).reshape(B, 1, N).astype(np.float32)
    M_out = M.reshape(B, 1, H)
    context = M.mean(axis=0, keepdims=True)  # [1, 512]
    logits = (context @ Wcls + bcls).astype(np.float32)  # [1, 2]
    ex = np.exp(logits - logits.max(axis=1, keepdims=True))
    Y_prob = (ex / ex.sum(axis=1, keepdims=True)).astype(np.float32)
    Y_hat = np.argmax(logits, axis=1, keepdims=True).astype(np.int32)
    return (logits, Y_prob, Y_hat, A_raw, M_out)
